# revision 1
# baseline (speedup 1.0000x reference)
"""Polyphase 2x upsample (scatter into one of 4 phases per batch) + circular
3x3 binomial blur, distributed over 8 TRN2 NeuronCores (data-parallel over
batch: 2 batches per core).

Math: with phase p per batch, r = p % 2, c = p // 2, the reference scatters
x[i,j] to y1[2i+r, 2j+c] (zeros elsewhere) and then blurs with
outer([1,2,1],[1,2,1])/16 under circular padding. The output decomposes into
4 parity classes (all indices mod 128, mod 64 inside a pair):
  out[2i+r,   2j+c]   = x[i,j] / 4                    (A sites)
  out[2i+r,   2k+1+c] = (x[i,k] + x[i,k+1]) / 8       (H sites)
  out[2i+1+r, 2j+c]   = (x[i,j] + x[i+1,j]) / 8       (V sites)
  out[2i+1+r, 2k+1+c] = sum of the 4 neighbours / 16  (D sites)
All multiplies are powers of two (exact in fp32). Memory-bound shifted-add:
the 40 MiB/core of HBM traffic (8 read + 32 write) is the roofline, so the
kernel is structured to keep the DMA rings busy end-to-end:
 - All four input-chunk loads are issued on SP before the offset-table load
   and values_load chain.
 - Per chunk, the four site classes are produced by two balanced engines
   (ACT: x16, x8 scaled copies, then A = 2*x8 and V = 2*Sv; DVE:
   Sv = x16 + rollrow(x16), then H = x8[k]+x8[k+1], D = Sv[k]+Sv[k+1]),
   so o2[0] completes ~14.7us after its input lands instead of ~17us+ for
   a serial t16->S->Sv->fused-VD chain. A reads x8 so x's last reader is
   the early x8 copy, freeing the input slot for the next prefetch early.

Hardware lessons baked in (measured on trn2):
 - tensor_tensor_reduce faults the runtime (CoreSim-only); use t16+adds.
 - GPSIMD software tensor ops contend with DVE for SBUF: concurrent Q7
   adds stall 150ns DVE ops to 3.5us. Pool engine does no compute here.
 - Strided-row DMA stores (per-row 512B descriptors) cost ~36% more HBM
   time than contiguous stores; all stores are contiguous row ranges.
 - A steady-state SWDGE (gpsimd) store can skew SDMA engine 15 slow (its
   AXI port also serves the SWDGE descriptor rings); all DMA is HWDGE.

SPMD phase handling (one NEFF for all 8 cores):
 - The column phase bit c selects between two fully static write layouts
   via a runtime 2-arm If per (batch, channel-half) — no per-instruction
   dynamic access patterns (each dynamic-AP instruction permanently burns
   ~2 registers on its engine, so they must stay rare).
 - The row shift by r is folded into the output DMA's DRAM row offsets
   (r, 64+r, (127+r)%128): fixed shapes, dynamic starts, loaded from a
   tiny per-core int32 input via values_load.
 - skip_runtime_bounds_check everywhere: the emitted software assert
   instruction faults this runtime.
"""

import sys

for _p in ("/opt/trn_rl_repo",):
    if _p not in sys.path:
        sys.path.insert(0, _p)

import numpy as np

B, C, N = 16, 256, 64
M = 2 * N
NCORES = 8
NB = B // NCORES  # batches per core

_NC_CACHE = None


def _build_nc():
    import concourse.bacc as bacc
    import concourse.bass as bass
    import concourse.mybir as mybir
    import concourse.tile as tile

    f32 = mybir.dt.float32
    i32 = mybir.dt.int32
    add = mybir.AluOpType.add
    ds = bass.ds
    ET = mybir.EngineType

    # Bacc (not plain Bass): its finalize() runs generate_event_semaphores,
    # which splits multi-wait instructions — this walrus build allows at
    # most one attached semaphore wait per instruction.
    nc = bacc.Bacc("TRN2", target_bir_lowering=False, debug=False, num_devices=NCORES)
    inp = nc.dram_tensor("inp", [NB, C, N, N], f32, kind="ExternalInput")
    offs = nc.dram_tensor("offs", [1, 16], i32, kind="ExternalInput")
    out = nc.dram_tensor("out", [NB, C, M, M], f32, kind="ExternalOutput")

    with tile.TileContext(nc) as tc:
        with (
            tc.tile_pool(name="offp", bufs=1) as offp,
            tc.tile_pool(name="xp", bufs=7) as xp,
            tc.tile_pool(name="x8p", bufs=1) as x8p,
            tc.tile_pool(name="t16p", bufs=1) as t16p,
            tc.tile_pool(name="svp", bufs=1) as svp,
            tc.tile_pool(name="op", bufs=2) as op,
        ):
            # Issue every input-chunk load first (SP queue) so HBM is busy
            # from t=0; nothing below can start until the first one lands
            # anyway.
            # Input streams in 16-row quarter-tiles: x is only read by
            # the two early ACT scaled copies, so small landing buffers cut
            # the x footprint (paying for double-buffered o tiles) while 7
            # slots keep ~2 chunks of input in flight — a late input piece
            # otherwise head-of-line-blocks the ACT queue at the batch
            # transition (~8us DMA gap).
            xs = {}
            for b in range(NB):
                for h in range(C // 128):
                    for j in range(4):
                        x = xp.tile(
                            [128, N // 4, N], f32, tag="x",
                            name=f"x_{b}_{h}_{j}",
                        )
                        nc.sync.dma_start(
                            x[:, :, :],
                            inp[b, 128 * h : 128 * (h + 1), 16 * j : 16 * j + 16],
                        )
                        xs[(b, h, j)] = x

            # Offset table on the ACT HWDGE queue so it doesn't delay SP.
            offs_t = offp.tile([1, 16], i32)
            nc.scalar.dma_start(offs_t[:, :], offs[:, :])

            # per batch: [cv, rv] at offs[0, 8*b + k]
            ranges = {
                "cv": (0, 1),    # c
                "rv": (0, 1),    # r
            }
            # cv selects the column layout (compute If on DVE+ACT); rv
            # selects the row layout (store If on SP). With both branches
            # fully static, every store AP is static: Tile can prove the
            # three stores of a chunk hit disjoint DRAM rows, so they
            # drain in parallel. (Dynamic ds() starts made the row
            # intervals conservatively overlap, serializing rA -> rB ->
            # rC per chunk — a ~20us serial store chain that set the
            # whole pipeline period.)
            engmap = {
                "cv": ((ET.DVE, ET.Activation),),
                "rv": ((ET.SP,),),
            }
            val = {}
            for b in range(NB):
                for k, name in enumerate(("cv", "rv")):
                    lo, hi = ranges[name]
                    for engs in engmap[name]:
                        val[(b, name, engs[0])] = nc.values_load(
                            offs_t[0:1, 8 * b + k : 8 * b + k + 1],
                            engines=list(engs),
                            min_val=lo,
                            max_val=hi,
                            skip_runtime_bounds_check=True,
                        )

            def writes(o, q, x8, Sv, c):
                """Phase-c static column layout for chunk q. Chunk rows are
                tile-relative: local even row 2i' holds A/H of input row
                32q+i', local odd row V/D of the row pair. A and V (scaled
                copies) on ACT; H and D (shifted adds) on DVE."""
                if c == 0:
                    a_cols = slice(0, 128, 2)        # A at cols 2j
                    hm_cols = slice(1, 127, 2)       # H at 2k+1: k=0..62
                    hw_col = 127                     # H wrap col (k=63)
                    v_cols = slice(0, 128, 2)        # V at cols 2j (no wrap)
                    dm_cols = slice(1, 127, 2)       # D at 2k+1: k=0..62
                    dw_col = 127                     # D wrap col (k=63)
                else:
                    a_cols = slice(1, 128, 2)        # A at cols 2j+1
                    hm_cols = slice(2, 127, 2)       # H at 2k+2: k=0..62
                    hw_col = 0                       # H wrap col (k=63)
                    v_cols = slice(1, 128, 2)        # V at cols 2j+1
                    dm_cols = slice(2, 127, 2)       # D at 2k+2: k=0..62
                    dw_col = 0                       # D wrap col (k=63)
                rs = slice(32 * q, 32 * q + 32)
                nrow = o.shape[1]  # 64 (q0 tile) or 63 (q1 main tile)
                er = slice(0, nrow, 2)       # even local rows (32)
                orr = slice(1, nrow, 2)      # odd local rows (32 or 31)
                n_odd = len(range(1, nrow, 2))
                vrs = slice(32 * q, 32 * q + n_odd)
                # A sites (ACT): even local rows. Read x8 (not x) so x's
                # last reader is the early x8 copy — frees the input slot
                # for the next prefetch ~8us sooner.
                nc.scalar.mul(o[:, er, a_cols], x8[:, rs, :], 2.0)
                # V sites (ACT): odd local rows, V = 2*Sv — all 64 cols,
                # the circular wrap is already folded into Sv.
                nc.scalar.mul(o[:, orr, v_cols], Sv[:, vrs, :], 2.0)
                # H sites (DVE): even local rows, H = x8[k]+x8[k+1]
                nc.vector.tensor_tensor(
                    o[:, er, hm_cols], x8[:, rs, 0:63], x8[:, rs, 1:64], add
                )
                nc.vector.tensor_tensor(
                    o[:, er, hw_col : hw_col + 1],
                    x8[:, rs, 63:64],
                    x8[:, rs, 0:1],
                    add,
                )
                # D sites (DVE): odd local rows, D = Sv[k]+Sv[k+1]
                nc.vector.tensor_tensor(
                    o[:, orr, dm_cols], Sv[:, vrs, 0:63], Sv[:, vrs, 1:64], add
                )
                nc.vector.tensor_tensor(
                    o[:, orr, dw_col : dw_col + 1],
                    Sv[:, vrs, 63:64],
                    Sv[:, vrs, 0:1],
                    add,
                )

            for b in range(NB):
                cv = val[(b, "cv", ET.DVE)]
                rv = val[(b, "rv", ET.SP)]
                for h in range(C // 128):
                    chs = slice(128 * h, 128 * (h + 1))

                    # x8 = x/8 feeds the H adds and A sites; t16 = x/16
                    # feeds Sv = (x+rollrow(x))/16 (V and D sites). Both
                    # are built per input half-tile as it lands.
                    t16 = t16p.tile([128, N, N], f32, tag="t16")
                    x8 = x8p.tile([128, N, N], f32, tag="x8")
                    for j in range(4):
                        xh = xs[(b, h, j)]
                        hr = slice(16 * j, 16 * j + 16)
                        nc.scalar.mul(t16[:, hr, :], xh[:, :, :], 0.0625)
                        nc.scalar.mul(x8[:, hr, :], xh[:, :, :], 0.125)
                    Sv = svp.tile([128, N, N], f32, tag="Sv")
                    nc.vector.tensor_tensor(
                        Sv[:, 0:63, :], t16[:, 0:63, :], t16[:, 1:64, :], add
                    )
                    nc.vector.tensor_tensor(
                        Sv[:, 63:64, :], t16[:, 63:64, :], t16[:, 0:1, :], add
                    )

                    out3 = out[b, chs]  # [128 ch, 128, 128] DRAM view
                    # Both o tiles are double-buffered (t16 lives in PSUM
                    # to make room): with a single buffer, the next
                    # chunk's site writes stall ~8us on the previous
                    # chunk's store drain (a DMAHW sem wait), which sets
                    # the whole pipeline period.
                    o0 = op.tile(
                        [128, 64, M], f32, tag="o0", name=f"o_{b}_{h}_0"
                    )
                    o1 = op.tile(
                        [128, 64, M], f32, tag="o1", name=f"o_{b}_{h}_1"
                    )
                    with tc.If(cv < 1) as cmp:
                        writes(o0, 0, x8, Sv, 0)
                        writes(o1, 1, x8, Sv, 0)
                    with cmp.Else():
                        writes(o0, 0, x8, Sv, 1)
                        writes(o1, 1, x8, Sv, 1)
                    # Contiguous stores (strided-row DMA stores cost ~36%
                    # more HBM time: per-row 512B descriptors). The row
                    # shift r picks one of two fully static layouts via an
                    # SP-side If, keeping every store AP static and
                    # provably disjoint so all three drain in parallel.
                    # All stores issue from SP: a store issue on the ACT
                    # queue waits on DVE's last site op and stalls the
                    # next chunk's t16/x8 behind it.
                    with tc.If(rv < 1) as smp:
                        nc.sync.dma_start(out3[:, 0:64, :], o0[:, :, :])
                        nc.sync.dma_start(out3[:, 127:128, :], o1[:, 63:64, :])
                        nc.sync.dma_start(out3[:, 64:127, :], o1[:, 0:63, :])
                    with smp.Else():
                        nc.sync.dma_start(out3[:, 1:65, :], o0[:, :, :])
                        nc.sync.dma_start(out3[:, 0:1, :], o1[:, 63:64, :])
                        nc.sync.dma_start(out3[:, 65:128, :], o1[:, 0:63, :])
    return nc


def _get_nc():
    global _NC_CACHE
    if _NC_CACHE is None:
        _NC_CACHE = _build_nc()
    return _NC_CACHE


def _offsets_for(idx_pair):
    offs = np.zeros((1, 16), np.int32)
    for j, p in enumerate(idx_pair):
        p = int(p)
        r, c = p % 2, p // 2
        offs[0, 8 * j : 8 * j + 4] = (c, r, 64 + r, (127 + r) % 128)
    return offs


def _to_np(a):
    if isinstance(a, np.ndarray):
        return a
    try:
        return np.asarray(a)
    except Exception:
        import jax

        return np.asarray(jax.device_put(a, jax.devices("cpu")[0]))


def kernel(inp, polyphase_indices, _trace=False):
    from concourse.bass_utils import run_bass_kernel_spmd

    inp = np.ascontiguousarray(_to_np(inp), dtype=np.float32)
    idx = _to_np(polyphase_indices).astype(np.int32).reshape(B)
    assert inp.shape == (B, C, N, N)

    in_maps = []
    for k in range(NCORES):
        in_maps.append(
            {
                "inp": np.ascontiguousarray(inp[NB * k : NB * (k + 1)]),
                "offs": _offsets_for(idx[NB * k : NB * (k + 1)]),
            }
        )

    nc = _get_nc()
    if not nc.is_finalized():
        nc.finalize()
    res = run_bass_kernel_spmd(
        nc, in_maps, core_ids=list(range(NCORES)), trace=_trace
    )
    out = np.concatenate([res.results[k]["out"] for k in range(NCORES)], axis=0)
    if _trace:
        kernel.last_results = res
    return out



# revision 13
# speedup vs baseline: 1.0032x; 1.0032x over previous
"""Polyphase 2x upsample (scatter into one of 4 phases per batch) + circular
3x3 binomial blur, distributed over 8 TRN2 NeuronCores (data-parallel over
batch: 2 batches per core).

Math: with phase p per batch, r = p % 2, c = p // 2, the reference scatters
x[i,j] to y1[2i+r, 2j+c] (zeros elsewhere) and then blurs with
outer([1,2,1],[1,2,1])/16 under circular padding. The output decomposes into
4 parity classes (all indices mod 128, mod 64 inside a pair):
  out[2i+r,   2j+c]   = x[i,j] / 4                    (A sites)
  out[2i+r,   2k+1+c] = (x[i,k] + x[i,k+1]) / 8       (H sites)
  out[2i+1+r, 2j+c]   = (x[i,j] + x[i+1,j]) / 8       (V sites)
  out[2i+1+r, 2k+1+c] = sum of the 4 neighbours / 16  (D sites)
All multiplies are powers of two (exact in fp32).

Memory-bound: 40 MiB/core of HBM traffic (8 read + 32 write). Stores cap at
~361 GB/s (the per-core HBM write limit) => 93.7us is the floor for the
store stream; the kernel is structured so that stream starts as early as
possible and never gaps:
 - Quarter-granularity software pipeline: each (batch, channel-half) chunk's
   128 output rows are produced in four ~32-row tiles (o_0..o_3), each
   stored the moment its sites complete. First store issues at ~11us
   (vs ~37us when the whole chunk must finish first), which fills the DMA
   hole between the end of the input-load stream and the old first store.
 - Queue split: SP issues ONLY stores (an earlier layout had 16 input-load
   issues, ~850ns each, queued ahead of the first store on SP). Input loads
   are issued from the ACT queue, one chunk ahead of the chunk being
   computed, so a load issue never blocks a store issue and lands well
   before use. xp bufs=8 (2 chunks) so a load issue's WAR wait always
   targets ops of chunk k-1 (already emitted), never ops behind it in its
   own queue.

Hardware lessons baked in (measured on trn2):
 - tensor_tensor_reduce faults the runtime (CoreSim-only); use adds.
 - GPSIMD software tensor ops contend with DVE for SBUF: concurrent Q7
   adds stall 150ns DVE ops to 3.5us. Pool engine does no compute here.
 - Strided-row DMA stores (per-row 512B descriptors) cost ~36% more HBM
   time than contiguous stores; all stores are contiguous row ranges.
 - HWDGE dma_start is only available on SP and ACT queues.

SPMD phase handling (one NEFF for all 8 cores):
 - The column phase bit c selects between two fully static write layouts
   via a runtime 2-arm If per chunk. All tiles are allocated OUTSIDE the
   If; both arms touch the same tiles with identical op counts (the
   baseline-proven pattern for Tile's cross-arm dependency accounting).
 - The row shift r is folded into the output DMA's DRAM row offsets via a
   2-arm If on SP: static starts in both arms, so Tile proves all stores
   of a chunk hit disjoint DRAM rows and they drain in parallel.
 - skip_runtime_bounds_check everywhere: the emitted software assert
   instruction faults this runtime.
"""

import sys

for _p in ("/opt/trn_rl_repo",):
    if _p not in sys.path:
        sys.path.insert(0, _p)

import numpy as np

B, C, N = 16, 256, 64
M = 2 * N
NCORES = 8
NB = B // NCORES  # batches per core

_NC_CACHE = None


def _build_nc():
    import concourse.bacc as bacc
    import concourse.bass as bass
    import concourse.mybir as mybir
    import concourse.tile as tile

    f32 = mybir.dt.float32
    i32 = mybir.dt.int32
    add = mybir.AluOpType.add
    ET = mybir.EngineType

    # Bacc (not plain Bass): its finalize() runs generate_event_semaphores,
    # which splits multi-wait instructions — this walrus build allows at
    # most one attached semaphore wait per instruction.
    nc = bacc.Bacc("TRN2", target_bir_lowering=False, debug=False, num_devices=NCORES)
    inp = nc.dram_tensor("inp", [NB, C, N, N], f32, kind="ExternalInput")
    offs = nc.dram_tensor("offs", [1, 16], i32, kind="ExternalInput")
    out = nc.dram_tensor("out", [NB, C, M, M], f32, kind="ExternalOutput")

    chunks = [(b, h) for b in range(NB) for h in range(C // 128)]

    with tile.TileContext(nc) as tc:
        # Pool-slot recycling must only happen ACROSS chunks (across
        # different Ifs): a slot whose release depends on readers inside an
        # If arm can only be reacquired by a later If's instructions (both
        # arms' reader accounting reconciles at the If merge). Reacquiring
        # within the same If deadlocks. Hence full-chunk x8/t16/Sv tiles
        # (quarters write disjoint row ranges of one tile) and xp bufs=8
        # (2 chunks), never recycled mid-If.
        with (
            tc.tile_pool(name="offp", bufs=1) as offp,
            tc.tile_pool(name="xp", bufs=8) as xp,
            tc.tile_pool(name="t16p", bufs=2) as t16p,
            tc.tile_pool(name="x8p", bufs=2) as x8p,
            tc.tile_pool(name="svp", bufs=2) as svp,
            tc.tile_pool(name="op", bufs=1) as op,
        ):
            def alloc_x(ci):
                b, h = chunks[ci]
                return [
                    xp.tile([128, 16, N], f32, tag="x", name=f"x_{b}_{h}_{j}")
                    for j in range(4)
                ]

            def issue_loads(ci, tiles):
                """Issue the 4 quarter-loads of chunk ci from the ACT queue."""
                b, h = chunks[ci]
                for j in range(4):
                    nc.scalar.dma_start(
                        tiles[j][:, :, :],
                        inp[b, 128 * h : 128 * (h + 1), 16 * j : 16 * j + 16],
                    )

            # Offset table first on the ACT HWDGE queue (tiny, lands while
            # the chunk-0 loads are being issued), then the chunk-0 loads,
            # THEN the values_loads — their reg-load instructions block the
            # ACT queue until the offs DMA lands, so the input loads must
            # already be in flight.
            offs_t = offp.tile([1, 16], i32)
            nc.scalar.dma_start(offs_t[:, :], offs[:, :])
            xs_cur = alloc_x(0)
            issue_loads(0, xs_cur)

            # per batch: [cv, rv] at offs[0, 8*b + k]
            val = {}
            for b in range(NB):
                for k, name, engs in (
                    (0, "cv", (ET.DVE, ET.Activation)),
                    (1, "rv", (ET.SP,)),
                ):
                    val[(b, name)] = nc.values_load(
                        offs_t[0:1, 8 * b + k : 8 * b + k + 1],
                        engines=list(engs),
                        min_val=0,
                        max_val=1,
                        skip_runtime_bounds_check=True,
                    )

            # Per-quarter output row groups (output row index before r shift):
            #   q0 -> rows [0,31)   : A/H at local even rows, V/D odd
            #   q1 -> rows [31,63)  : V/D at local even rows, A/H odd
            #   q2 -> rows [63,95)  : V/D even, A/H odd
            #   q3 -> rows [95,128) : V/D even, A/H odd, local row 32 = pair 63
            # Quarter j's A/H rows read x8 rows [16j,16j+16); its V/D rows
            # read Sv pairs (q0: [0,15), q1: [15,31), q2: [31,47),
            # q3: [47,63) plus the wrap pair 63 at Sv row 63).
            def compute_chunk(ci, xs, t16, x8, Sv, os, c, prefetch):
                if c == 0:
                    a_cols = slice(0, 128, 2)
                    hm_cols = slice(1, 127, 2)
                    hw_col = 127
                    v_cols = slice(0, 128, 2)
                    dm_cols = slice(1, 127, 2)
                    dw_col = 127
                else:
                    a_cols = slice(1, 128, 2)
                    hm_cols = slice(2, 127, 2)
                    hw_col = 0
                    v_cols = slice(1, 128, 2)
                    dm_cols = slice(2, 127, 2)
                    dw_col = 0
                for j in range(4):
                    if j == 2 and prefetch is not None:
                        # mid-chunk prefetch of the next chunk's loads: far
                        # enough in that the issue's WAR wait (chunk ci-1's
                        # copies) is long satisfied, early enough that the
                        # data lands well before chunk ci+1's copies.
                        issue_loads(ci + 1, prefetch)
                    xq, o = xs[j], os[j]
                    hr = slice(16 * j, 16 * j + 16)
                    # t16 = x/16 feeds Sv; x8 = x/8 feeds A and H.
                    nc.scalar.mul(t16[:, hr, :], xq[:, :, :], 0.0625)
                    nc.scalar.mul(x8[:, hr, :], xq[:, :, :], 0.125)
                    # Sv pairs needed by this quarter's V/D rows
                    if j == 0:
                        pr = slice(0, 15)
                        ah = slice(0, 31, 2)   # 16 rows
                        vd = slice(1, 30, 2)   # 15 rows
                    else:
                        pr = slice(16 * j - 1, 16 * j + 15)
                        ah = slice(1, 32, 2)   # 16 rows
                        vd = slice(0, 31, 2)   # 16 rows
                    nc.vector.tensor_tensor(
                        Sv[:, pr, :],
                        t16[:, pr, :],
                        t16[:, pr.start + 1 : pr.stop + 1, :],
                        add,
                    )
                    if j == 3:
                        nc.vector.tensor_tensor(
                            Sv[:, 63:64, :], t16[:, 63:64, :], t16[:, 0:1, :], add
                        )
                    # ACT: A = 2*x8, V = 2*Sv (scaled copies)
                    nc.scalar.mul(o[:, ah, a_cols], x8[:, hr, :], 2.0)
                    nc.scalar.mul(o[:, vd, v_cols], Sv[:, pr, :], 2.0)
                    # DVE: H = x8[k]+x8[k+1], D = Sv[k]+Sv[k+1]
                    nc.vector.tensor_tensor(
                        o[:, ah, hm_cols], x8[:, hr, 0:63], x8[:, hr, 1:64], add
                    )
                    nc.vector.tensor_tensor(
                        o[:, ah, hw_col : hw_col + 1],
                        x8[:, hr, 63:64],
                        x8[:, hr, 0:1],
                        add,
                    )
                    nc.vector.tensor_tensor(
                        o[:, vd, dm_cols], Sv[:, pr, 0:63], Sv[:, pr, 1:64], add
                    )
                    nc.vector.tensor_tensor(
                        o[:, vd, dw_col : dw_col + 1],
                        Sv[:, pr, 63:64],
                        Sv[:, pr, 0:1],
                        add,
                    )
                    if j == 3:
                        # wrap row (pair 63) at local row 32 of o_3
                        wr = slice(32, 33)
                        pw = slice(63, 64)
                        nc.scalar.mul(o[:, wr, v_cols], Sv[:, pw, :], 2.0)
                        nc.vector.tensor_tensor(
                            o[:, wr, dm_cols], Sv[:, pw, 0:63], Sv[:, pw, 1:64], add
                        )
                        nc.vector.tensor_tensor(
                            o[:, wr, dw_col : dw_col + 1],
                            Sv[:, pw, 63:64],
                            Sv[:, pw, 0:1],
                            add,
                        )

            # o-tile row spans (before r shift): q0 31 rows, q1/q2 32, q3 33.
            O_ROWS = (31, 32, 32, 33)

            for ci in range(len(chunks)):
                b, h = chunks[ci]
                xs = xs_cur
                xs_next = alloc_x(ci + 1) if ci + 1 < len(chunks) else None
                t16 = t16p.tile([128, N, N], f32, tag="t16")
                x8 = x8p.tile([128, N, N], f32, tag="x8", name=f"x8_{b}_{h}")
                Sv = svp.tile([128, N, N], f32, tag="sv", name=f"sv_{b}_{h}")
                os = [
                    op.tile([128, O_ROWS[j], M], f32, tag=f"o{j}", name=f"o_{b}_{h}_{j}")
                    for j in range(4)
                ]
                cv = val[(b, "cv")]
                with tc.If(cv < 1) as cmp:
                    compute_chunk(ci, xs, t16, x8, Sv, os, 0, xs_next)
                with cmp.Else():
                    compute_chunk(ci, xs, t16, x8, Sv, os, 1, xs_next)
                xs_cur = xs_next

                out3 = out[b, 128 * h : 128 * (h + 1)]  # [128ch, 128, 128]
                rv = val[(b, "rv")]
                # Contiguous-row stores; static APs in both arms so Tile
                # proves row-disjointness and the stores drain in parallel.
                with tc.If(rv < 1) as smp:
                    nc.sync.dma_start(out3[:, 0:31, :], os[0][:, :, :])
                    nc.sync.dma_start(out3[:, 31:63, :], os[1][:, :, :])
                    nc.sync.dma_start(out3[:, 63:95, :], os[2][:, :, :])
                    nc.sync.dma_start(out3[:, 95:128, :], os[3][:, :, :])
                with smp.Else():
                    nc.sync.dma_start(out3[:, 1:32, :], os[0][:, :, :])
                    nc.sync.dma_start(out3[:, 32:64, :], os[1][:, :, :])
                    nc.sync.dma_start(out3[:, 64:96, :], os[2][:, :, :])
                    nc.sync.dma_start(out3[:, 96:128, :], os[3][:, 0:32, :])
                    nc.sync.dma_start(out3[:, 0:1, :], os[3][:, 32:33, :])
    return nc


def _get_nc():
    global _NC_CACHE
    if _NC_CACHE is None:
        _NC_CACHE = _build_nc()
    return _NC_CACHE


def _offsets_for(idx_pair):
    offs = np.zeros((1, 16), np.int32)
    for j, p in enumerate(idx_pair):
        p = int(p)
        r, c = p % 2, p // 2
        offs[0, 8 * j : 8 * j + 4] = (c, r, 64 + r, (127 + r) % 128)
    return offs


def _to_np(a):
    if isinstance(a, np.ndarray):
        return a
    try:
        return np.asarray(a)
    except Exception:
        import jax

        return np.asarray(jax.device_put(a, jax.devices("cpu")[0]))


def kernel(inp, polyphase_indices, _trace=False):
    from concourse.bass_utils import run_bass_kernel_spmd

    inp = np.ascontiguousarray(_to_np(inp), dtype=np.float32)
    idx = _to_np(polyphase_indices).astype(np.int32).reshape(B)
    assert inp.shape == (B, C, N, N)

    in_maps = []
    for k in range(NCORES):
        in_maps.append(
            {
                "inp": np.ascontiguousarray(inp[NB * k : NB * (k + 1)]),
                "offs": _offsets_for(idx[NB * k : NB * (k + 1)]),
            }
        )

    nc = _get_nc()
    if not nc.is_finalized():
        nc.finalize()
    res = run_bass_kernel_spmd(
        nc, in_maps, core_ids=list(range(NCORES)), trace=_trace
    )
    out = np.concatenate([res.results[k]["out"] for k in range(NCORES)], axis=0)
    if _trace:
        kernel.last_results = res
    return out
